# revision 22
# baseline (speedup 1.0000x reference)
"""Distributed GAT kernel for one TRN2 chip (8 NeuronCores).

Math (matches reference.py):
  layer(h, W, a1, a2):  Wh = h@W;  f1 = Wh@a1;  f2 = Wh@a2
    e = leaky_relu(f1_i + f2_j, 0.2);  att = softmax(where(adj, e, -inf), j)
    out = att @ Wh
  Key factorization used on-device: with z = f1_i + f2_j,
    exp(leaky_relu(z) - f1_i) = max(exp(f2_j), exp(0.2 f2_j) * exp(-0.8 f1_i))
  so the softmax numerator is p_ij = m_ij * max(a_j, b_j * c_i) with
  a = exp(f2), b = exp(0.2 f2), c = exp(-0.8 f1): no transcendentals on the
  [N, N] tiles, and the row-constant exp(-f1_i) cancels in the softmax.

Sharding: row-parallel over query nodes (512 rows/core). Each core computes
Wh for its own 512 nodes, all-gathers Wh + f2 (bf16/f32), and runs its
[512, 4096] slice of both attention layers against the shared mask kept
resident in SBUF as bf16. log-softmax sums and the MLP head partial sums
are all-reduced; every core computes the identical [64, 256] output.
"""

import os
import sys

import numpy as np

for _p in ("/opt/trn_rl_repo", "/root/.axon_site/_ro/trn_rl_repo"):
    if os.path.isdir(_p) and _p not in sys.path:
        sys.path.insert(0, _p)

import ml_dtypes  # noqa: E402
import concourse.bass as bass  # noqa: E402
import concourse.bacc as bacc  # noqa: E402
import concourse.tile as tile  # noqa: E402
from concourse import mybir  # noqa: E402
from concourse.bass_utils import run_bass_kernel_spmd  # noqa: E402

F = mybir.ActivationFunctionType
OP = mybir.AluOpType
BF = mybir.dt.bfloat16
F32 = mybir.dt.float32
I32 = mybir.dt.int32

N, FIN, H, D, C = 4096, 512, 8, 64, 64
NC = 8            # cores
NB = N // NC      # 512 query rows per core
P = 128
CH = N // P       # 32 key chunks
ICH = NB // P     # 4 local row chunks
GRP = 4           # key chunks per tensor_tensor mask op
MLP_H = 512       # W1 output dim
FCH = MLP_H // P  # 4


def build():
    nc = bacc.Bacc("TRN2", target_bir_lowering=False, debug=False, num_devices=NC)

    # ---- per-core external inputs (host-sharded / host-transposed layouts) ----
    adjT = nc.dram_tensor("adjT", [N, NB], I32, kind="ExternalInput")       # adj[block,:].T
    xT = nc.dram_tensor("xT", [FIN, NB], F32, kind="ExternalInput")         # x[block,:].T
    w_d = nc.dram_tensor("w", [H, FIN, D], F32, kind="ExternalInput")
    a1_d = nc.dram_tensor("a1", [H, D], F32, kind="ExternalInput")
    a2_d = nc.dram_tensor("a2", [H, D], F32, kind="ExternalInput")
    wo_d = nc.dram_tensor("wo", [FIN, D], F32, kind="ExternalInput")
    ao1_d = nc.dram_tensor("ao1", [D], F32, kind="ExternalInput")
    ao2_d = nc.dram_tensor("ao2", [D], F32, kind="ExternalInput")
    w1t_d = nc.dram_tensor("w1t", [NB, MLP_H], F32, kind="ExternalInput")   # W1[:, block].T
    b1_d = nc.dram_tensor("b1", [P, FCH], F32, kind="ExternalInput")        # b1.reshape(4,128).T
    w2t_d = nc.dram_tensor("w2t", [MLP_H, 256], F32, kind="ExternalInput")  # W2.T
    b2_d = nc.dram_tensor("b2", [P, 2], F32, kind="ExternalInput")          # b2.reshape(2,128).T
    out_d = nc.dram_tensor("out", [C, 256], F32, kind="ExternalOutput")

    idn_d = nc.inline_tensor(np.eye(P, dtype=ml_dtypes.bfloat16), name="idn")
    idn32_d = nc.inline_tensor(np.eye(P, dtype=np.float32), name="idn32")

    # ---- internal DRAM for collectives ----
    wh_in = nc.dram_tensor("wh_in", [H, P, ICH, D + 1], BF)
    f2_in = nc.dram_tensor("f2_in", [H, P, ICH], F32)
    wh_out = nc.dram_tensor("wh_out", [NC, H, P, ICH, D + 1], BF, addr_space="Shared")
    f2_out = nc.dram_tensor("f2_out", [NC, H, P, ICH], F32, addr_space="Shared")
    who_in = nc.dram_tensor("who_in", [P, ICH, D + 1], BF)
    f2o_in = nc.dram_tensor("f2o_in", [P, ICH], F32)
    who_out = nc.dram_tensor("who_out", [NC, P, ICH, D + 1], BF, addr_space="Shared")
    f2o_out = nc.dram_tensor("f2o_out", [NC, P, ICH], F32, addr_space="Shared")
    lsm_in = nc.dram_tensor("lsm_in", [C, 1], F32)
    lsm_out = nc.dram_tensor("lsm_out", [C, 1], F32, addr_space="Shared")
    p1_in = nc.dram_tensor("p1_in", [P, FCH, D], F32)
    p1_out = nc.dram_tensor("p1_out", [P, FCH, D], F32, addr_space="Shared")

    GROUPS = [list(range(NC))]

    with tile.TileContext(nc) as tc:
        with (
            tc.tile_pool(name="const", bufs=1) as pc,
            tc.tile_pool(name="stage", bufs=1) as pst,
            tc.tile_pool(name="adjstage", bufs=2) as pad,
            tc.tile_pool(name="work", bufs=3) as pw,
            tc.tile_pool(name="tail", bufs=2) as pt,
            tc.tile_pool(name="mlp", bufs=1) as pm,
            tc.tile_pool(name="head", bufs=1) as ph,
            tc.tile_pool(name="headwork", bufs=2) as ph2,
            tc.tile_pool(name="gath", bufs=2) as pg,
            tc.tile_pool(name="psout", bufs=2, space="PSUM") as ppo,
            tc.tile_pool(name="psmisc", bufs=3, space="PSUM") as ppm,
        ):
            # ================= phase 0: constants =================
            ones1 = pc.tile([1, P], BF)
            nc.vector.memset(ones1[:], 1.0)
            idn = pc.tile([P, P], BF)
            nc.gpsimd.dma_start(idn[:], idn_d[:])
            idn32 = pc.tile([P, P], F32)
            nc.gpsimd.dma_start(idn32[:], idn32_d[:])

            def load_bf(dram_ap, shape, name_dims):
                st = pst.tile(shape, F32, tag="stage_" + name_dims)
                nc.gpsimd.dma_start(st[:], dram_ap)
                bf_t = pc.tile(shape, BF, tag=name_dims)
                nc.vector.tensor_copy(bf_t[:], st[:])
                return bf_t

            # xT: [FIN, NB] -> [128, ICH(f), NB]  (xcT[p, fc, i] = xT[fc*128+p, i])
            xcT = load_bf(xT[:].rearrange("(fc p) i -> p fc i", p=P), [P, FCH, NB], "xcT")
            # W: [H, FIN, D] -> [128, H, FCH, D]  (h-major so the DMA AP merges to 3D)
            w_st = pst.tile([P, H, FCH, D], F32, tag="stage_w")
            nc.gpsimd.dma_start(w_st[:], w_d[:].rearrange("h (fc p) d -> p h fc d", p=P))
            w_bf = pc.tile([P, H, FCH, D], BF, tag="w_bf")
            nc.vector.tensor_copy(w_bf[:], w_st[:])
            wo_bf = load_bf(wo_d[:].rearrange("(fc p) d -> p fc d", p=P), [P, FCH, D], "wo_bf")
            w1t_f = pc.tile([P, ICH, MLP_H], F32, tag="w1t")
            nc.gpsimd.dma_start(w1t_f[:], w1t_d[:].rearrange("(ic p) f -> p ic f", p=P))
            w2t_f = pc.tile([P, FCH, 256], F32, tag="w2t")
            nc.gpsimd.dma_start(w2t_f[:], w2t_d[:].rearrange("(fc p) o -> p fc o", p=P))

            # attention vectors: a12[d, k, 0] = a1[k, d]; a12[d, k, 1] = a2[k, d]
            a12 = pc.tile([D, H, 2], F32)
            nc.gpsimd.dma_start(a12[:, :, 0:1], a1_d[:].rearrange("h (d one) -> d h one", one=1))
            nc.gpsimd.dma_start(a12[:, :, 1:2], a2_d[:].rearrange("h (d one) -> d h one", one=1))
            ao12 = pc.tile([D, 2], F32)
            nc.gpsimd.dma_start(ao12[:, 0:1], ao1_d[:].rearrange("(d one) -> d one", one=1))
            nc.gpsimd.dma_start(ao12[:, 1:2], ao2_d[:].rearrange("(d one) -> d one", one=1))

            # a2 rows (for broadcast tiles): a2r[0, k*64+d] = a2[k, d]
            a2r = pc.tile([1, H * D], F32)
            nc.gpsimd.dma_start(a2r[:], a2_d[:].rearrange("(one h) d -> one (h d)", one=1))
            a2r_bf = pc.tile([1, H * D], BF)
            nc.vector.tensor_copy(a2r_bf[:], a2r[:])
            ao2r = pc.tile([1, D], F32)
            nc.gpsimd.dma_start(ao2r[:], ao2_d[:].rearrange("(one d) -> one d", one=1))
            ao2r_bf = pc.tile([1, D], BF)
            nc.vector.tensor_copy(ao2r_bf[:], ao2r[:])

            b1c = pc.tile([P, FCH], F32)
            nc.gpsimd.dma_start(b1c[:], b1_d[:])
            b2c = pc.tile([P, 2], F32)
            nc.gpsimd.dma_start(b2c[:], b2_d[:])

            # a2 broadcast tiles [128, H*D] via PE outer product
            ps_a2bc = ppm.tile([P, H * D], F32, tag="pmisc")
            nc.tensor.matmul(ps_a2bc[:], ones1[:], a2r_bf[:], start=True, stop=True)
            a2bc = pc.tile([P, H * D], BF)
            nc.scalar.copy(a2bc[:], ps_a2bc[:])
            ps_ao2bc = ppm.tile([P, D], F32, tag="pmisc")
            nc.tensor.matmul(ps_ao2bc[:], ones1[:], ao2r_bf[:], start=True, stop=True)
            ao2bc = pc.tile([P, D], BF)
            nc.scalar.copy(ao2bc[:], ps_ao2bc[:])

            # ============ phase 0b: mask -> bf16, transposed-resident ============
            mT = pc.tile([P, CH, NB], BF)
            for ch4 in range(CH // 4):
                st = pad.tile([P, 4, NB], I32, tag="adj")
                nc.gpsimd.dma_start(
                    st[:], adjT[bass.ts(ch4, 4 * P), :].rearrange("(c p) i -> p c i", p=P)
                )
                nc.gpsimd.tensor_copy(mT[:, bass.ts(ch4, 4), :], st[:])

            # ============ phase 1: per-head Wh / f2 + gather ============
            cbc_list = []
            for k in range(H):
                # Wh rows for this core's nodes: [128, ic, D]
                whc = ph2.tile([P, ICH, D + 1], BF, tag="whc")
                nc.vector.memset(whc[:, :, D : D + 1], 1.0)
                for ic in range(ICH):
                    ps = ppm.tile([P, D], F32, tag="pmisc")
                    for fc in range(FCH):
                        nc.tensor.matmul(
                            ps[:], xcT[:, fc, bass.ts(ic, P)], w_bf[:, k, fc, :],
                            start=(fc == 0), stop=(fc == FCH - 1),
                        )
                    nc.scalar.copy(whc[:, ic, 0:D], ps[:])
                nc.sync.dma_start(wh_in[k], whc[:])

                # Wh^T [D, NB] f32 (for f-rows)
                ps_whT = ppm.tile([D, NB], F32, tag="pmisc")
                for fc in range(FCH):
                    nc.tensor.matmul(
                        ps_whT[:], w_bf[:, k, fc, :], xcT[:, fc, :],
                        start=(fc == 0), stop=(fc == FCH - 1),
                    )
                whT_f = ph2.tile([D, NB], F32, tag="whT")
                nc.scalar.copy(whT_f[:], ps_whT[:])
                # f rows: [2, NB] = [a1|a2]^T @ Wh^T  (fp32 matmul)
                ps_f = ppm.tile([2, NB], F32, tag="pmisc")
                nc.tensor.matmul(ps_f[:], a12[:, k, :], whT_f[:], start=True, stop=True)
                # c row = exp(-0.8 f1) -> bf16
                crow = ph2.tile([1, NB], BF, tag="crow")
                nc.scalar.activation(crow[:], ps_f[0:1, :], F.Exp, scale=-0.8)
                # C broadcast tile [128, NB]
                ps_cbc = ppm.tile([P, NB], F32, tag="pmisc")
                nc.tensor.matmul(ps_cbc[:], ones1[:], crow[:], start=True, stop=True)
                cbc = ph.tile([P, NB], BF, tag=f"cbc{k}")  # all 8 live past the gather
                nc.scalar.copy(cbc[:], ps_cbc[:])
                cbc_list.append(cbc)

                # f2 columns [128, ICH] via fused mult+reduce against a2 broadcast
                f2c = ph2.tile([P, ICH], F32, tag="f2c")
                for ic in range(ICH):
                    junk = pt.tile([P, D], F32, tag="ttrjunk")
                    nc.vector.tensor_tensor(junk[:], whc[:, ic, 0:D], a2bc[:, bass.ts(k, D)], OP.mult)
                    nc.vector.tensor_reduce(f2c[:, ic : ic + 1], junk[:], mybir.AxisListType.X, OP.add)
                nc.sync.dma_start(f2_in[k], f2c[:])

            nc.gpsimd.collective_compute(
                "AllGather", OP.bypass, replica_groups=GROUPS,
                ins=[wh_in[:]], outs=[wh_out[:]],
            )
            nc.gpsimd.collective_compute(
                "AllGather", OP.bypass, replica_groups=GROUPS,
                ins=[f2_in[:]], outs=[f2_out[:]],
            )

            # gathered f2 columns (tiny, resident for all heads)
            f2g = pc.tile([P, H, CH], F32)
            for k in range(H):
                for c in range(NC):
                    nc.sync.dma_start(f2g[:, k, bass.ts(c, ICH)], f2_out[c, k])

            # ============ phase 2: layer-1 attention ============
            hT = pc.tile([P, FCH, NB], BF)  # h^T, assembled from head outputs

            def attention(whg_k, f2g_k, cbc, psum_pool, tag):
                """Runs masked attention for one head; returns psum [D+1, NB]."""
                a_sb = pt.tile([P, CH], F32, tag="a_sb")
                b_sb = pt.tile([P, CH], F32, tag="b_sb")
                nc.scalar.activation(a_sb[:], f2g_k, F.Exp)
                nc.scalar.activation(b_sb[:], f2g_k, F.Exp, scale=0.2)
                ps_o = psum_pool.tile([D + 1, NB], F32, tag="att_out")
                for g in range(CH // GRP):
                    t4 = pw.tile([P, GRP, NB], BF, tag="t4")
                    for q in range(GRP):
                        cc = g * GRP + q
                        nc.vector.tensor_scalar(
                            t4[:, q, :], cbc[:], b_sb[:, cc : cc + 1],
                            a_sb[:, cc : cc + 1], OP.mult, OP.max,
                        )
                    p4 = pw.tile([P, GRP, NB], BF, tag="p4")
                    nc.vector.tensor_tensor(
                        p4[:], t4[:], mT[:, bass.ts(g, GRP), :], OP.mult
                    )
                    for q in range(GRP):
                        cc = g * GRP + q
                        nc.tensor.matmul(
                            ps_o[:], whg_k[:, cc, :], p4[:, q, :],
                            start=(cc == 0), stop=(cc == CH - 1),
                        )
                return ps_o

            def finish_softmax(ps_o, out_bf, tag):
                """out_bf[D, NB] (bf16, SBUF) = elu(ps_o[0:D] / ps_o[D])."""
                zinv = pt.tile([1, NB], F32, tag="zi")
                nc.vector.reciprocal(zinv[:], ps_o[D : D + 1, :])
                zinv_bf = pt.tile([1, NB], BF, tag="zib")
                nc.scalar.copy(zinv_bf[:], zinv[:])
                ps_z = ppm.tile([D, NB], F32, tag="pmisc")
                nc.tensor.matmul(ps_z[:], ones1[0:1, 0:D], zinv_bf[:], start=True, stop=True)
                zbc = pt.tile([D, NB], F32, tag="zbc")
                nc.scalar.copy(zbc[:], ps_z[:])
                u = pt.tile([D, NB], BF, tag="u")
                nc.vector.tensor_tensor(u[:], ps_o[0:D, :], zbc[:], OP.mult)
                # elu(u) = min(exp(u) - 1, relu(u))
                e = pt.tile([D, NB], BF, tag="e")
                nc.scalar.activation(e[:], u[:], F.Exp)
                r = pt.tile([D, NB], BF, tag="r")
                nc.scalar.activation(r[:], u[:], F.Relu)
                e1 = pt.tile([D, NB], BF, tag="e1")
                nc.vector.tensor_scalar(e1[:], e[:], 1.0, None, OP.subtract)
                nc.vector.tensor_tensor(out_bf, e1[:], r[:], OP.min)

            for k in range(H):
                whg_k = pg.tile([P, CH, D + 1], BF, tag="whg")
                for c in range(NC):
                    nc.sync.dma_start(whg_k[:, bass.ts(c, ICH), :], wh_out[c, k])
                ps_o = attention(whg_k[:], f2g[:, k, :], cbc_list[k][:], ppo, f"h{k}")
                finish_softmax(ps_o, hT[bass.ts(k % 2, D), k // 2, :], f"l1_{k % 2}")

            # ============ phase 3: layer-2 Wh_o + gather ============
            whoc = pc.tile([P, ICH, D + 1], BF)
            nc.vector.memset(whoc[:, :, D : D + 1], 1.0)
            for ic in range(ICH):
                ps = ppm.tile([P, D], F32, tag="pmisc")
                for fc in range(FCH):
                    nc.tensor.matmul(
                        ps[:], hT[:, fc, bass.ts(ic, P)], wo_bf[:, fc, :],
                        start=(fc == 0), stop=(fc == FCH - 1),
                    )
                nc.scalar.copy(whoc[:, ic, 0:D], ps[:])
            nc.sync.dma_start(who_in[:], whoc[:])

            ps_whoT = ppm.tile([D, NB], F32, tag="pmisc")
            for fc in range(FCH):
                nc.tensor.matmul(
                    ps_whoT[:], wo_bf[:, fc, :], hT[:, fc, :],
                    start=(fc == 0), stop=(fc == FCH - 1),
                )
            whoT_f = pc.tile([D, NB], F32)
            nc.scalar.copy(whoT_f[:], ps_whoT[:])
            ps_fo = ppm.tile([2, NB], F32, tag="pmisc")
            nc.tensor.matmul(ps_fo[:], ao12[:], whoT_f[:], start=True, stop=True)
            crow2 = pc.tile([1, NB], BF)
            nc.scalar.activation(crow2[:], ps_fo[0:1, :], F.Exp, scale=-0.8)
            ps_cbc2 = ppm.tile([P, NB], F32, tag="pmisc")
            nc.tensor.matmul(ps_cbc2[:], ones1[:], crow2[:], start=True, stop=True)
            cbc2 = pc.tile([P, NB], BF)
            nc.scalar.copy(cbc2[:], ps_cbc2[:])
            f2oc = pc.tile([P, ICH], F32)
            for ic in range(ICH):
                junk2 = pt.tile([P, D], F32, tag="ttrjunk")
                nc.vector.tensor_tensor(junk2[:], whoc[:, ic, 0:D], ao2bc[:], OP.mult)
                nc.vector.tensor_reduce(f2oc[:, ic : ic + 1], junk2[:], mybir.AxisListType.X, OP.add)
            nc.sync.dma_start(f2o_in[:], f2oc[:])

            nc.gpsimd.collective_compute(
                "AllGather", OP.bypass, replica_groups=GROUPS,
                ins=[who_in[:]], outs=[who_out[:]],
            )
            nc.gpsimd.collective_compute(
                "AllGather", OP.bypass, replica_groups=GROUPS,
                ins=[f2o_in[:]], outs=[f2o_out[:]],
            )
            whg2 = pc.tile([P, CH, D + 1], BF)
            f2og = pc.tile([P, CH], F32)
            for c in range(NC):
                nc.sync.dma_start(whg2[:, bass.ts(c, ICH), :], who_out[c])
                nc.sync.dma_start(f2og[:, bass.ts(c, ICH)], f2o_out[c])

            # ============ phase 4: layer-2 attention ============
            ps_o2 = attention(whg2[:], f2og[:], cbc2[:], ppo, "l2")
            o_sb = pc.tile([D, NB], F32)
            finish_softmax(ps_o2, o_sb[:], "l2")

            # ============ phase 5: log-softmax over nodes ============
            ejunk = pm.tile([D, NB], BF, tag="ejunk")
            esum = pm.tile([D, 1], F32, tag="esum")
            nc.scalar.activation(ejunk[:], o_sb[:], F.Exp, accum_out=esum[:])
            nc.sync.dma_start(lsm_in[:], esum[:])
            nc.gpsimd.collective_compute(
                "AllReduce", OP.add, replica_groups=GROUPS,
                ins=[lsm_in[:]], outs=[lsm_out[:]],
            )
            ssum = pm.tile([D, 1], F32, tag="ssum")
            nc.sync.dma_start(ssum[:], lsm_out[:])
            logs = pm.tile([D, 1], F32, tag="logs")
            nc.scalar.activation(logs[:], ssum[:], F.Ln)
            o2 = pc.tile([D, NB], F32)
            nc.vector.tensor_scalar(o2[:], o_sb[:], logs[:], None, OP.subtract)

            # ============ phase 6: MLP head ============
            # transpose o2 -> o chunks [128, D]
            och = pc.tile([P, ICH, D], F32)
            for ic in range(ICH):
                ps_t = ppm.tile([P, D], F32, tag="ptr")
                nc.tensor.transpose(ps_t[:], o2[:, bass.ts(ic, P)], idn32[0:D, 0:D])
                nc.scalar.copy(och[:, ic, :], ps_t[:])
            # partial^T[f, c] = sum_i W1T[i, f] o[i, c]
            p1 = pm.tile([P, FCH, D], F32, tag="p1")
            for ft in range(FCH):
                ps_p = ppm.tile([P, D], F32, tag="pmisc")
                for ic in range(ICH):
                    nc.tensor.matmul(
                        ps_p[:], w1t_f[:, ic, bass.ts(ft, P)], och[:, ic, :],
                        start=(ic == 0), stop=(ic == ICH - 1),
                    )
                nc.scalar.copy(p1[:, ft, :], ps_p[:])
            nc.sync.dma_start(p1_in[:], p1[:])
            nc.gpsimd.collective_compute(
                "AllReduce", OP.add, replica_groups=GROUPS,
                ins=[p1_in[:]], outs=[p1_out[:]],
            )
            p1g = pm.tile([P, FCH, D], F32, tag="p1g")
            nc.sync.dma_start(p1g[:], p1_out[:])
            # y1^T = leaky_relu(partial + b1, 0.1)
            y1 = pm.tile([P, FCH, D], F32, tag="y1")
            y1s = pm.tile([P, FCH, D], F32, tag="y1s")
            y1t = pc.tile([P, FCH, D], F32)
            for ft in range(FCH):
                nc.scalar.activation(y1[:, ft, :], p1g[:, ft, :], F.Identity, bias=b1c[:, ft : ft + 1])
            nc.vector.tensor_scalar(y1s[:], y1[:], 0.1, None, OP.mult)
            nc.vector.tensor_tensor(y1t[:], y1[:], y1s[:], OP.max)
            # y2^T[o, c] = W2T^T @ y1^T + b2
            y2 = pm.tile([P, 2, D], F32, tag="y2")
            for oh in range(2):
                ps_y = ppm.tile([P, D], F32, tag="pmisc")
                for ft in range(FCH):
                    nc.tensor.matmul(
                        ps_y[:], w2t_f[:, ft, bass.ts(oh, P)], y1t[:, ft, :],
                        start=(ft == 0), stop=(ft == FCH - 1),
                    )
                nc.scalar.activation(y2[:, oh, :], ps_y[:], F.Identity, bias=b2c[:, oh : oh + 1])
            # transpose back: y[c, o]
            y_sb = pm.tile([D, 256], F32, tag="ysb")
            for oh in range(2):
                ps_t = ppm.tile([D, P], F32, tag="ptr")
                nc.tensor.transpose(ps_t[:], y2[:, oh, :], idn32[:])
                nc.scalar.copy(y_sb[:, bass.ts(oh, P)], ps_t[:])
            nc.sync.dma_start(out_d[:], y_sb[:])

    nc.compile()
    return nc


_NC_CACHE = {}


def _get_nc():
    if "nc" not in _NC_CACHE:
        _NC_CACHE["nc"] = build()
    return _NC_CACHE["nc"]


def make_in_maps(x, adj, W, a1, a2, Wo, ao1, ao2, W1, b1, W2, b2):
    x = np.asarray(x, np.float32)
    adj = np.ascontiguousarray(np.asarray(adj, np.int32))
    W1 = np.asarray(W1, np.float32)
    shared = {
        "w": np.ascontiguousarray(np.asarray(W, np.float32)),
        "a1": np.ascontiguousarray(np.asarray(a1, np.float32)),
        "a2": np.ascontiguousarray(np.asarray(a2, np.float32)),
        "wo": np.ascontiguousarray(np.asarray(Wo, np.float32)),
        "ao1": np.ascontiguousarray(np.asarray(ao1, np.float32)),
        "ao2": np.ascontiguousarray(np.asarray(ao2, np.float32)),
        "w2t": np.ascontiguousarray(np.asarray(W2, np.float32).T),
        "b1": np.ascontiguousarray(np.asarray(b1, np.float32).reshape(FCH, P).T),
        "b2": np.ascontiguousarray(np.asarray(b2, np.float32).reshape(2, P).T),
    }
    in_maps = []
    for c in range(NC):
        blk = slice(c * NB, (c + 1) * NB)
        in_maps.append({
            "adjT": np.ascontiguousarray(adj[blk, :].T),
            "xT": np.ascontiguousarray(x[blk, :].T),
            "w1t": np.ascontiguousarray(W1[:, blk].T),
            **shared,
        })
    return in_maps


def kernel(**inputs):
    nc = _get_nc()
    in_maps = make_in_maps(**inputs)
    res = run_bass_kernel_spmd(nc, in_maps, list(range(NC)))
    return np.asarray(res.results[0]["out"], np.float32)


if __name__ == "__main__":
    rng = np.random.default_rng(0)
    ins = {
        "x": rng.normal(size=(N, FIN)).astype(np.float32),
        "adj": rng.integers(0, 2, size=(N, N)).astype(np.int32),
        "W": (rng.normal(size=(H, FIN, D)) * 0.1).astype(np.float32),
        "a1": (rng.normal(size=(H, D)) * 0.1).astype(np.float32),
        "a2": (rng.normal(size=(H, D)) * 0.1).astype(np.float32),
        "Wo": (rng.normal(size=(H * D, C)) * 0.1).astype(np.float32),
        "ao1": (rng.normal(size=(C,)) * 0.1).astype(np.float32),
        "ao2": (rng.normal(size=(C,)) * 0.1).astype(np.float32),
        "W1": (rng.normal(size=(MLP_H, N)) * 0.01).astype(np.float32),
        "b1": np.zeros(MLP_H, np.float32),
        "W2": (rng.normal(size=(256, MLP_H)) * 0.04).astype(np.float32),
        "b2": np.zeros(256, np.float32),
    }
    out = kernel(**ins)
    print("out", out.shape, out.dtype, np.abs(out).mean())


# revision 32
# speedup vs baseline: 1.0023x; 1.0023x over previous
"""Distributed GAT kernel for one TRN2 chip (8 NeuronCores).

Math (matches reference.py):
  layer(h, W, a1, a2):  Wh = h@W;  f1 = Wh@a1;  f2 = Wh@a2
    e = leaky_relu(f1_i + f2_j, 0.2);  att = softmax(where(adj, e, -inf), j)
    out = att @ Wh
  Key factorization used on-device: with z = f1_i + f2_j,
    exp(leaky_relu(z) - f1_i) = max(exp(f2_j), exp(0.2 f2_j) * exp(-0.8 f1_i))
  so the softmax numerator is p_ij = m_ij * max(a_j, b_j * c_i) with
  a = exp(f2), b = exp(0.2 f2), c = exp(-0.8 f1): no transcendentals on the
  [N, N] tiles, and the row-constant exp(-f1_i) cancels in the softmax.

Sharding: row-parallel over query nodes (512 rows/core). Each core computes
Wh for its own 512 nodes; Wh, a softmax-ones column, and the f2 bits are
packed into one AllGather payload per layer. Each core then runs its
[512, 4096] slice of both attention layers against the shared adjacency
mask kept resident in SBUF as bf16. The log-softmax enters the MLP head as
an exact rank-1 correction (colsum(W1T) x -log S) so the MLP matmuls can
run in bf16 on the uncentered activations; log-softmax sums and MLP
partial sums are all-reduced. Every core computes the identical [64, 256]
output.
"""

import os
import sys

import numpy as np

for _p in ("/opt/trn_rl_repo", "/root/.axon_site/_ro/trn_rl_repo"):
    if os.path.isdir(_p) and _p not in sys.path:
        sys.path.insert(0, _p)

import ml_dtypes  # noqa: E402
import concourse.bass as bass  # noqa: E402
import concourse.bacc as bacc  # noqa: E402
import concourse.tile as tile  # noqa: E402
from concourse import mybir  # noqa: E402
from concourse.bass_utils import run_bass_kernel_spmd  # noqa: E402

F = mybir.ActivationFunctionType
OP = mybir.AluOpType
BF = mybir.dt.bfloat16
F32 = mybir.dt.float32
I32 = mybir.dt.int32
U16 = mybir.dt.uint16

N, FIN, H, D, C = 4096, 512, 8, 64, 64
NC = 8            # cores
NB = N // NC      # 512 query rows per core
P = 128
CH = N // P       # 32 key chunks
ICH = NB // P     # 4 local row chunks
GRP = 8           # key chunks per work group
MLP_H = 512       # W1 output dim
FCH = MLP_H // P  # 4
SCOL = D + 1                    # stationary columns (Wh | ones)
NCOL = ICH * SCOL + 2 * ICH    # + f2 (f32 as 2 bf16 slots per value)
F2OFF = ICH * SCOL             # 260: byte offset 520, 4B-aligned


USE_ACT_PATH = True  # knob: ACT-assisted t computation


def _use_act(gi):
    """Group-level engine split for the t = b*c computation."""
    return USE_ACT_PATH and gi % 4 != 0


def build(debug_taps=False):
    nc = bacc.Bacc("TRN2", target_bir_lowering=False, debug=False, num_devices=NC)

    # ---- per-core external inputs (host-sharded / host-transposed layouts) ----
    adjT = nc.dram_tensor("adjT", [N, NB], I32, kind="ExternalInput")       # adj[block,:].T
    xT = nc.dram_tensor("xT", [FIN, NB], F32, kind="ExternalInput")         # x[block,:].T
    w_d = nc.dram_tensor("w", [H, FIN, D], F32, kind="ExternalInput")
    a1_d = nc.dram_tensor("a1", [H, D], F32, kind="ExternalInput")
    a2_d = nc.dram_tensor("a2", [H, D], F32, kind="ExternalInput")
    wo_d = nc.dram_tensor("wo", [FIN, D], F32, kind="ExternalInput")
    ao1_d = nc.dram_tensor("ao1", [D], F32, kind="ExternalInput")
    ao2_d = nc.dram_tensor("ao2", [D], F32, kind="ExternalInput")
    w1t_d = nc.dram_tensor("w1t", [NB, MLP_H], F32, kind="ExternalInput")   # W1[:, block].T
    b1_d = nc.dram_tensor("b1", [P, FCH], F32, kind="ExternalInput")        # b1.reshape(4,128).T
    w2t_d = nc.dram_tensor("w2t", [MLP_H, 256], F32, kind="ExternalInput")  # W2.T
    b2_d = nc.dram_tensor("b2", [P, 2], F32, kind="ExternalInput")          # b2.reshape(2,128).T
    out_d = nc.dram_tensor("out", [C, 256], F32, kind="ExternalOutput")
    taps = {}
    if debug_taps:
        for nm, shp, dt in [("t_hT", [P, FCH, NB], BF), ("t_whoc", [P, NCOL], BF),
                            ("t_osb", [D, NB], BF), ("t_p1", [P, FCH, SCOL], F32),
                            ("t_esum", [D, 1], F32), ("t_ssum", [D, 1], F32),
                            ("t_y1t", [P, FCH, D], BF), ("t_cbc0", [P, NB], BF),
                            ("t_whg0", [P, NC, NCOL], BF), ("t_ab0", [P, NC, ICH], F32)]:
            taps[nm] = nc.dram_tensor(nm, shp, dt, kind="ExternalOutput")

    idn_d = nc.inline_tensor(np.eye(P, dtype=ml_dtypes.bfloat16), name="idn")
    idn32_d = nc.inline_tensor(np.eye(P, dtype=np.float32), name="idn32")

    # ---- internal DRAM for collectives ----
    wh_in = nc.dram_tensor("wh_in", [H, P, NCOL], BF)
    wh_out = nc.dram_tensor("wh_out", [NC, H, P, NCOL], BF, addr_space="Shared")
    who_in = nc.dram_tensor("who_in", [P, NCOL], BF)
    who_out = nc.dram_tensor("who_out", [NC, P, NCOL], BF, addr_space="Shared")
    lsm_in = nc.dram_tensor("lsm_in", [C, 1], F32)
    lsm_out = nc.dram_tensor("lsm_out", [C, 1], F32, addr_space="Shared")
    p1_in = nc.dram_tensor("p1_in", [P, FCH, D + 1], F32)
    p1_out = nc.dram_tensor("p1_out", [P, FCH, D + 1], F32, addr_space="Shared")

    GROUPS = [list(range(NC))]

    with tile.TileContext(nc) as tc:
        with (
            tc.tile_pool(name="const", bufs=1) as pc,
            tc.tile_pool(name="stage", bufs=1) as pst,
            tc.tile_pool(name="adjstage", bufs=2) as pad,
            tc.tile_pool(name="work", bufs=2) as pw,
            tc.tile_pool(name="tail", bufs=2) as pt,
            tc.tile_pool(name="mlp", bufs=1) as pm,
            tc.tile_pool(name="head", bufs=1) as ph,
            tc.tile_pool(name="headwork", bufs=2) as ph2,
            tc.tile_pool(name="gath", bufs=2) as pg,
            tc.tile_pool(name="psout", bufs=2, space="PSUM") as ppo,
            tc.tile_pool(name="psmisc", bufs=3, space="PSUM") as ppm,
            tc.tile_pool(name="pstr", bufs=2, space="PSUM") as ppt,
            tc.tile_pool(name="psp1", bufs=1, space="PSUM") as pp1,
        ):
            # ================= phase 0: constants =================
            ones1 = pc.tile([1, P], BF)
            nc.vector.memset(ones1[:], 1.0)
            ones128 = pc.tile([P, 1], BF)
            nc.vector.memset(ones128[:], 1.0)
            idn = pc.tile([P, P], BF)
            nc.sync.dma_start(idn[:], idn_d[:])
            idn32 = pc.tile([P, P], F32)
            nc.sync.dma_start(idn32[:], idn32_d[:])

            def load_bf(dram_ap, shape, name_dims):
                st = pst.tile(shape, F32, tag="stage_" + name_dims)
                nc.sync.dma_start(st[:], dram_ap)
                bf_t = pc.tile(shape, BF, tag=name_dims)
                nc.vector.tensor_copy(bf_t[:], st[:])
                return bf_t

            xcT = load_bf(xT[:].rearrange("(fc p) i -> p fc i", p=P), [P, FCH, NB], "xcT")
            # W: [H, FIN, D] -> [128, H, FCH, D]  (h-major so the DMA AP merges to 3D)
            w_st = pst.tile([P, H, FCH, D], F32, tag="stage_w")
            nc.sync.dma_start(w_st[:], w_d[:].rearrange("h (fc p) d -> p h fc d", p=P))
            w_bf = pc.tile([P, H, FCH, D], BF, tag="w_bf")
            nc.vector.tensor_copy(w_bf[:], w_st[:])
            wo_bf = load_bf(wo_d[:].rearrange("(fc p) d -> p fc d", p=P), [P, FCH, D], "wo_bf")
            w1t_st = pst.tile([P, ICH, MLP_H], F32, tag="stage_w1t")
            nc.sync.dma_start(w1t_st[:], w1t_d[:].rearrange("(ic p) f -> p ic f", p=P))
            w1t_bf = pc.tile([P, ICH, MLP_H], BF, tag="w1t")
            nc.vector.tensor_copy(w1t_bf[:], w1t_st[:])
            w2t_bf = load_bf(w2t_d[:].rearrange("(fc p) o -> p fc o", p=P), [P, FCH, 256], "w2t")

            # attention vectors: a12[d, k, 0] = a1[k, d]; a12[d, k, 1] = a2[k, d]
            a12 = pst.tile([D, H, 2], F32, tag="stage_a12")
            nc.sync.dma_start(a12[:, :, 0:1], a1_d[:].rearrange("h (d one) -> d h one", one=1))
            nc.sync.dma_start(a12[:, :, 1:2], a2_d[:].rearrange("h (d one) -> d h one", one=1))
            a12_bf = pc.tile([D, H, 2], BF)
            nc.vector.tensor_copy(a12_bf[:], a12[:])
            ao12 = pst.tile([D, 2], F32, tag="stage_ao12")
            nc.sync.dma_start(ao12[:, 0:1], ao1_d[:].rearrange("(d one) -> d one", one=1))
            nc.sync.dma_start(ao12[:, 1:2], ao2_d[:].rearrange("(d one) -> d one", one=1))
            ao12_bf = pc.tile([D, 2], BF)
            nc.vector.tensor_copy(ao12_bf[:], ao12[:])

            a2r = pst.tile([1, H * D], F32, tag="stage_a2r")
            nc.sync.dma_start(a2r[:], a2_d[:].rearrange("(one h) d -> one (h d)", one=1))
            a2r_bf = pc.tile([1, H * D], BF)
            nc.vector.tensor_copy(a2r_bf[:], a2r[:])
            ao2r = pst.tile([1, D], F32, tag="stage_ao2r")
            nc.sync.dma_start(ao2r[:], ao2_d[:].rearrange("(one d) -> one d", one=1))
            ao2r_bf = pc.tile([1, D], BF)
            nc.vector.tensor_copy(ao2r_bf[:], ao2r[:])

            b1c = pc.tile([P, FCH], F32)
            nc.sync.dma_start(b1c[:], b1_d[:])
            b2c = pc.tile([P, 2], F32)
            nc.sync.dma_start(b2c[:], b2_d[:])

            # a2 broadcast tiles via PE outer product
            ps_a2bc = ppm.tile([P, H * D], F32, tag="pmisc")
            nc.tensor.matmul(ps_a2bc[:], ones1[:], a2r_bf[:], start=True, stop=True)
            a2bc = pc.tile([P, H * D], BF)
            nc.scalar.copy(a2bc[:], ps_a2bc[:])
            ps_ao2bc = ppm.tile([P, D], F32, tag="pmisc")
            nc.tensor.matmul(ps_ao2bc[:], ones1[:], ao2r_bf[:], start=True, stop=True)
            ao2bc = pc.tile([P, D], BF)
            nc.scalar.copy(ao2bc[:], ps_ao2bc[:])

            # ---- mask DMA loads start now (scalar HWDGE queue, 8 x 1 MiB) ----
            mT = pc.tile([P, CH, NB], BF)
            adj_stages = []
            for ch4 in range(CH // 4):
                st = pad.tile([P, 4, NB], I32, tag="adj")
                nc.scalar.dma_start(
                    st[:], adjT[bass.ts(ch4, 4 * P), :].rearrange("(c p) i -> p c i", p=P)
                )
                adj_stages.append(st)

            def cast_mask(ch4):
                nc.gpsimd.tensor_copy(mT[:, bass.ts(ch4, 4), :], adj_stages[ch4][:])

            # first 8 chunks converted before the first collective hits the
            # gpsimd queue; the rest convert behind it (during the gather)
            cast_mask(0)
            cast_mask(1)

            # ============ phase 1: per-head Wh / f-rows / packed payload ============
            cbc_list = []
            for k in range(H):
                ps_whT = ppm.tile([D, NB], F32, tag="pmisc")
                for fc in range(FCH):
                    nc.tensor.matmul(
                        ps_whT[:], w_bf[:, k, fc, :], xcT[:, fc, :],
                        start=(fc == 0), stop=(fc == FCH - 1),
                    )
                whT_bf = ph2.tile([D, NB], BF, tag="whT")
                nc.scalar.copy(whT_bf[:], ps_whT[:])
                ps_f = ppm.tile([2, NB], F32, tag="pmisc")
                nc.tensor.matmul(ps_f[:], a12_bf[:, k, :], whT_bf[:], start=True, stop=True)
                crow = ph2.tile([1, NB], BF, tag="crow")
                nc.scalar.activation(crow[:], ps_f[0:1, :], F.Exp, scale=-0.8)
                ps_cbc = ppm.tile([P, NB], F32, tag="pmisc")
                nc.tensor.matmul(ps_cbc[:], ones1[:], crow[:], start=True, stop=True)
                cbc = ph.tile([P, NB], BF, tag=f"cbc{k}")  # all 8 live past the gather
                nc.scalar.copy(cbc[:], ps_cbc[:])
                cbc_list.append(cbc)

                # packed payload: [Wh|1] x ICH + f2 bits
                whc = ph2.tile([P, NCOL], BF, tag="whc")
                whcv = whc[:, 0 : F2OFF].rearrange("p (ic c) -> p ic c", c=SCOL)
                nc.vector.memset(whcv[:, :, D : D + 1], 1.0)
                f2c = ph2.tile([P, ICH], F32, tag="f2c")
                for ic in range(ICH):
                    ps_t = ppt.tile([P, D], BF, tag="ptr")
                    nc.tensor.transpose(ps_t[:], whT_bf[:, bass.ts(ic, P)], idn[0:D, 0:D])
                    nc.scalar.copy(whcv[:, ic, 0:D], ps_t[:])
                    junk = pt.tile([P, D], F32, tag="ttrjunk")
                    nc.vector.tensor_tensor(
                        junk[:], whcv[:, ic, 0:D], a2bc[:, bass.ts(k, D)], OP.mult
                    )
                    nc.vector.tensor_reduce(
                        f2c[:, ic : ic + 1], junk[:], mybir.AxisListType.X, OP.add
                    )
                # raw bit move (f32 <-> bf16 views both as u16: no conversion)
                nc.vector.tensor_copy(
                    whc[:, F2OFF:NCOL].bitcast(U16), f2c[:].bitcast(U16)
                )
                nc.sync.dma_start(wh_in[k], whc[:])

            nc.gpsimd.collective_compute(
                "AllGather", OP.bypass, replica_groups=GROUPS,
                ins=[wh_in[:]], outs=[wh_out[:]],
            )
            for ch4 in range(2, CH // 4):
                cast_mask(ch4)

            # ============ phase 2: attention (shared by both layers) ============
            hT = pc.tile([P, FCH, NB], BF)  # h^T, assembled from head outputs

            def attention(whg_k, cbc, gbase):
                """Masked attention for one head; returns psum [D+1, NB].

                whg_k: [P, NC, NCOL] gathered payload (Wh|1 stationaries + f2
                bits). p = mask * max(a_j, b_j * c_i), numerator and Z from one
                accumulated matmul chain against [Wh | 1].
                """
                f2t = pt.tile([P, NC, ICH], F32, tag="f2t")
                nc.vector.tensor_copy(
                    f2t[:].bitcast(U16), whg_k[:, :, F2OFF:NCOL].bitcast(U16)
                )
                a_sb = pt.tile([P, NC, ICH], F32, tag="a_sb")
                b_sb = pt.tile([P, NC, ICH], F32, tag="b_sb")
                nc.scalar.activation(a_sb[:], f2t[:], F.Exp)
                nc.scalar.activation(b_sb[:], f2t[:], F.Exp, scale=0.2)
                ps_o = ppo.tile([D + 1, NB], F32, tag="att_out")
                for g in range(CH // GRP):
                    use_act = _use_act(gbase + g)
                    t8 = pw.tile([P, GRP, NB], BF, tag="t8")
                    p8 = pw.tile([P, GRP, NB], BF, tag="p8")
                    for q in range(GRP):
                        cc = g * GRP + q
                        c_, chl = cc // ICH, cc % ICH
                        b_sl = b_sb[:, c_, chl : chl + 1]
                        a_sl = a_sb[:, c_, chl : chl + 1]
                        if use_act:
                            nc.scalar.activation(
                                t8[:, q, :], cbc[:], F.Identity, bias=0.0, scale=b_sl
                            )
                            nc.vector.scalar_tensor_tensor(
                                p8[:, q, :], t8[:, q, :], a_sl, mT[:, cc, :],
                                OP.max, OP.mult,
                            )
                        else:
                            nc.vector.tensor_scalar(
                                t8[:, q, :], cbc[:], b_sl, a_sl, OP.mult, OP.max
                            )
                    if not use_act:
                        nc.vector.tensor_tensor(
                            p8[:], t8[:], mT[:, bass.ts(g, GRP), :], OP.mult
                        )
                    for q in range(GRP):
                        cc = g * GRP + q
                        c_, chl = cc // ICH, cc % ICH
                        nc.tensor.matmul(
                            ps_o[:], whg_k[:, c_, chl * SCOL : chl * SCOL + SCOL],
                            p8[:, q, :],
                            start=(cc == 0), stop=(cc == CH - 1),
                        )
                return ps_o

            def finish_softmax(ps_o, out_bf):
                """out_bf[D, NB] (bf16, SBUF) = elu(ps_o[0:D] / ps_o[D])."""
                lnz = pt.tile([1, NB], F32, tag="zi")
                nc.scalar.activation(lnz[:], ps_o[D : D + 1, :], F.Ln)
                zinv_bf = pt.tile([1, NB], BF, tag="zib")
                nc.scalar.activation(zinv_bf[:], lnz[:], F.Exp, scale=-1.0)
                ps_z = ppm.tile([D, NB], F32, tag="pmisc")
                nc.tensor.matmul(ps_z[:], ones1[0:1, 0:D], zinv_bf[:], start=True, stop=True)
                zbc = pt.tile([D, NB], F32, tag="zbc")
                nc.scalar.copy(zbc[:], ps_z[:])
                u = pt.tile([D, NB], BF, tag="u")
                nc.vector.tensor_tensor(u[:], ps_o[0:D, :], zbc[:], OP.mult)
                # elu(u) = min(exp(u) - 1, relu(u))
                e = pt.tile([D, NB], BF, tag="e")
                nc.scalar.activation(e[:], u[:], F.Exp)
                r = pt.tile([D, NB], BF, tag="r")
                nc.scalar.activation(r[:], u[:], F.Relu)
                nc.vector.scalar_tensor_tensor(out_bf, e[:], -1.0, r[:], OP.add, OP.min)

            for k in range(H):
                whg_k = pg.tile([P, NC, NCOL], BF, tag="whg")
                for c in range(NC):
                    nc.sync.dma_start(whg_k[:, c, :], wh_out[c, k])
                if taps and k == 0:
                    nc.sync.dma_start(taps["t_whg0"][:], whg_k[:])
                ps_o = attention(whg_k[:], cbc_list[k], k * (CH // GRP))
                finish_softmax(ps_o, hT[bass.ts(k % 2, D), k // 2, :])
                if taps and k == 0:
                    nc.sync.dma_start(taps["t_cbc0"][:], cbc_list[0][:])

            # ============ phase 3: layer-2 Wh_o + gather ============
            whoc = pc.tile([P, NCOL], BF)
            whocv = whoc[:, 0 : F2OFF].rearrange("p (ic c) -> p ic c", c=SCOL)
            nc.vector.memset(whocv[:, :, D : D + 1], 1.0)
            f2oc = pm.tile([P, ICH], F32, tag="f2oc")
            for ic in range(ICH):
                ps = ppm.tile([P, D], F32, tag="pmisc")
                for fc in range(FCH):
                    nc.tensor.matmul(
                        ps[:], hT[:, fc, bass.ts(ic, P)], wo_bf[:, fc, :],
                        start=(fc == 0), stop=(fc == FCH - 1),
                    )
                nc.scalar.copy(whocv[:, ic, 0:D], ps[:])
                junk2 = pt.tile([P, D], F32, tag="ttrjunk")
                nc.vector.tensor_tensor(junk2[:], whocv[:, ic, 0:D], ao2bc[:], OP.mult)
                nc.vector.tensor_reduce(
                    f2oc[:, ic : ic + 1], junk2[:], mybir.AxisListType.X, OP.add
                )
            nc.vector.tensor_copy(
                whoc[:, F2OFF:NCOL].bitcast(U16), f2oc[:].bitcast(U16)
            )
            nc.sync.dma_start(who_in[:], whoc[:])
            if taps:
                nc.sync.dma_start(taps["t_hT"][:], hT[:])
                nc.sync.dma_start(taps["t_whoc"][:], whoc[:])

            ps_whoT = ppm.tile([D, NB], F32, tag="pmisc")
            for fc in range(FCH):
                nc.tensor.matmul(
                    ps_whoT[:], wo_bf[:, fc, :], hT[:, fc, :],
                    start=(fc == 0), stop=(fc == FCH - 1),
                )
            whoT_bf = pc.tile([D, NB], BF)
            nc.scalar.copy(whoT_bf[:], ps_whoT[:])
            ps_fo = ppm.tile([2, NB], F32, tag="pmisc")
            nc.tensor.matmul(ps_fo[:], ao12_bf[:], whoT_bf[:], start=True, stop=True)
            crow2 = pc.tile([1, NB], BF)
            nc.scalar.activation(crow2[:], ps_fo[0:1, :], F.Exp, scale=-0.8)
            ps_cbc2 = ppm.tile([P, NB], F32, tag="pmisc")
            nc.tensor.matmul(ps_cbc2[:], ones1[:], crow2[:], start=True, stop=True)
            cbc2 = pc.tile([P, NB], BF)
            nc.scalar.copy(cbc2[:], ps_cbc2[:])

            nc.gpsimd.collective_compute(
                "AllGather", OP.bypass, replica_groups=GROUPS,
                ins=[who_in[:]], outs=[who_out[:]],
            )
            whg2 = pc.tile([P, NC, NCOL], BF)
            for c in range(NC):
                nc.sync.dma_start(whg2[:, c, :], who_out[c])

            # ============ phase 4: layer-2 attention ============
            ps_o2 = attention(whg2[:], cbc2, H * (CH // GRP))
            o_sb = pc.tile([D, NB], BF)
            finish_softmax(ps_o2, o_sb[:])

            # ============ phase 5: log-softmax sum (overlaps MLP term-2) ======
            ejunk = pm.tile([D, NB], BF, tag="ejunk")
            esum = pm.tile([D, 1], F32, tag="esum")
            nc.scalar.activation(ejunk[:], o_sb[:], F.Exp, accum_out=esum[:])
            nc.sync.dma_start(lsm_in[:], esum[:])
            if taps:
                nc.sync.dma_start(taps["t_osb"][:], o_sb[:])
                nc.sync.dma_start(taps["t_esum"][:], esum[:])
            nc.gpsimd.collective_compute(
                "AllReduce", OP.add, replica_groups=GROUPS,
                ins=[lsm_in[:]], outs=[lsm_out[:]],
            )

            # ============ phase 6: MLP head ============
            # fp32 column sums of W1T (scaled by log S later -> needs f32)
            ones128f = pm.tile([P, 1], F32, tag="ones128f")
            nc.vector.memset(ones128f[:], 1.0)
            ps_csr = ppm.tile([1, MLP_H], F32, tag="pmisc")
            for ic in range(ICH):
                nc.tensor.matmul(
                    ps_csr[:], ones128f[:], w1t_st[:, ic, :],
                    start=(ic == 0), stop=(ic == ICH - 1),
                )
            csr = pm.tile([1, MLP_H], F32, tag="csr")
            nc.scalar.copy(csr[:], ps_csr[:])
            cs_cols = pm.tile([P, FCH], F32, tag="cscols")
            for ft in range(FCH):
                ps_c = ppt.tile([P, 1], F32, tag="ptr")
                nc.tensor.transpose(ps_c[:], csr[0:1, bass.ts(ft, P)], idn32[0:1, 0:1])
                nc.scalar.copy(cs_cols[:, ft : ft + 1], ps_c[:])
            # o chunks [128, D] (transpose of o_sb)
            och = pm.tile([P, ICH, D], BF, tag="och")
            for ic in range(ICH):
                ps_t = ppt.tile([P, D], BF, tag="ptr")
                nc.tensor.transpose(ps_t[:], o_sb[:, bass.ts(ic, P)], idn[0:D, 0:D])
                nc.scalar.copy(och[:, ic, :], ps_t[:])
            # term 2: [f, 0:64] = sum_i W1T[i, f] o[c, i]  (pre-centering)
            ps_p1 = pp1.tile([P, FCH * D], F32, tag="p1ps")
            for ft in range(FCH):
                for ic in range(ICH):
                    nc.tensor.matmul(
                        ps_p1[:, bass.ts(ft, D)],
                        w1t_bf[:, ic, bass.ts(ft, P)], och[:, ic, :],
                        start=(ic == 0), stop=(ic == ICH - 1),
                    )
            p1sb = pm.tile([P, FCH, SCOL], F32, tag="p1")
            nc.scalar.copy(
                p1sb[:, :, 0:D],
                ps_p1[:].rearrange("p (ft d) -> p ft d", d=D),
            )
            nc.vector.tensor_copy(
                p1sb[:, :, D : D + 1],
                cs_cols[:].rearrange("p (ft one) -> p ft one", one=1),
            )
            nc.sync.dma_start(p1_in[:], p1sb[:])
            if taps:
                nc.sync.dma_start(taps["t_p1"][:], p1sb[:])
            nc.gpsimd.collective_compute(
                "AllReduce", OP.add, replica_groups=GROUPS,
                ins=[p1_in[:]], outs=[p1_out[:]],
            )
            p1g = pm.tile([P, FCH, SCOL], F32, tag="p1g")
            nc.sync.dma_start(p1g[:], p1_out[:])
            # -log S broadcast tile [C, ...] -> [128, D] via PE outer
            ssum = pm.tile([D, 1], F32, tag="ssum")
            nc.sync.dma_start(ssum[:], lsm_out[:])
            logs = pm.tile([D, 1], F32, tag="logs")
            nc.scalar.activation(logs[:], ssum[:], F.Ln)
            nls_col = pm.tile([D, 1], F32, tag="nlsc")
            nc.scalar.activation(nls_col[:], logs[:], F.Copy, scale=-1.0)
            ps_nr = ppt.tile([1, D], F32, tag="ptr")
            nc.tensor.transpose(ps_nr[:], nls_col[:], idn32[0:D, 0:D])
            nls_row = pm.tile([1, D], F32, tag="nlsr")
            nc.scalar.copy(nls_row[:], ps_nr[:])
            ones1f = pm.tile([1, P], F32, tag="ones1f")
            nc.vector.memset(ones1f[:], 1.0)
            ps_nb = ppm.tile([P, D], F32, tag="pmisc")
            nc.tensor.matmul(ps_nb[:], ones1f[:], nls_row[:], start=True, stop=True)
            nls_bc = pm.tile([P, D], F32, tag="nlsbc")
            nc.scalar.copy(nls_bc[:], ps_nb[:])
            # y1^T = leaky_relu(term2 + colsum * (-log S) + b1, 0.1)
            y1t = pm.tile([P, FCH, D], BF, tag="y1t")
            for ft in range(FCH):
                v0 = pm.tile([P, D], F32, tag="v0")
                nc.vector.scalar_tensor_tensor(
                    v0[:], nls_bc[:], p1g[:, ft, D : D + 1], p1g[:, ft, 0:D],
                    OP.mult, OP.add,
                )
                v = pm.tile([P, D], F32, tag="v")
                nc.scalar.activation(
                    v[:], v0[:], F.Identity, bias=b1c[:, ft : ft + 1]
                )
                nc.vector.scalar_tensor_tensor(
                    y1t[:, ft, :], v[:], 0.1, v[:], OP.mult, OP.max
                )
            if taps:
                nc.sync.dma_start(taps["t_ssum"][:], ssum[:])
                nc.sync.dma_start(taps["t_y1t"][:], y1t[:])
            # y2^T[o, c] = W2T^T @ y1^T + b2
            y2 = pm.tile([P, 2, D], F32, tag="y2")
            for oh in range(2):
                ps_y = ppm.tile([P, D], F32, tag="pmisc")
                for ft in range(FCH):
                    nc.tensor.matmul(
                        ps_y[:], w2t_bf[:, ft, bass.ts(oh, P)], y1t[:, ft, :],
                        start=(ft == 0), stop=(ft == FCH - 1),
                    )
                nc.scalar.activation(
                    y2[:, oh, :], ps_y[:], F.Identity, bias=b2c[:, oh : oh + 1]
                )
            # transpose back: y[c, o]
            y_sb = pm.tile([D, 256], F32, tag="ysb")
            for oh in range(2):
                ps_t = ppt.tile([D, P], F32, tag="ptr")
                nc.tensor.transpose(ps_t[:], y2[:, oh, :], idn32[:])
                nc.scalar.copy(y_sb[:, bass.ts(oh, P)], ps_t[:])
            nc.sync.dma_start(out_d[:], y_sb[:])

    nc.compile()
    return nc


_NC_CACHE = {}


def _get_nc():
    if "nc" not in _NC_CACHE:
        _NC_CACHE["nc"] = build()
    return _NC_CACHE["nc"]


def make_in_maps(x, adj, W, a1, a2, Wo, ao1, ao2, W1, b1, W2, b2):
    x = np.asarray(x, np.float32)
    adj = np.ascontiguousarray(np.asarray(adj, np.int32))
    W1 = np.asarray(W1, np.float32)
    shared = {
        "w": np.ascontiguousarray(np.asarray(W, np.float32)),
        "a1": np.ascontiguousarray(np.asarray(a1, np.float32)),
        "a2": np.ascontiguousarray(np.asarray(a2, np.float32)),
        "wo": np.ascontiguousarray(np.asarray(Wo, np.float32)),
        "ao1": np.ascontiguousarray(np.asarray(ao1, np.float32)),
        "ao2": np.ascontiguousarray(np.asarray(ao2, np.float32)),
        "w2t": np.ascontiguousarray(np.asarray(W2, np.float32).T),
        "b1": np.ascontiguousarray(np.asarray(b1, np.float32).reshape(FCH, P).T),
        "b2": np.ascontiguousarray(np.asarray(b2, np.float32).reshape(2, P).T),
    }
    in_maps = []
    for c in range(NC):
        blk = slice(c * NB, (c + 1) * NB)
        in_maps.append({
            "adjT": np.ascontiguousarray(adj[blk, :].T),
            "xT": np.ascontiguousarray(x[blk, :].T),
            "w1t": np.ascontiguousarray(W1[:, blk].T),
            **shared,
        })
    return in_maps


def kernel(**inputs):
    nc = _get_nc()
    in_maps = make_in_maps(**inputs)
    res = run_bass_kernel_spmd(nc, in_maps, list(range(NC)))
    return np.asarray(res.results[0]["out"], np.float32)


if __name__ == "__main__":
    import reference as R  # only for standalone smoke runs

    ins = {k: np.asarray(v) for k, v in R.setup_inputs().items()}
    out = kernel(**ins)
    print("out", out.shape, out.dtype, np.abs(out).mean())
